# revision 1
# baseline (speedup 1.0000x reference)
"""NT-Xent loss Trainium2 kernel (8-core SPMD, Bass/Tile).

Math: loss = mean_a [ log(den_a) - pos_a/tau ],
  den_a = sum_{b != a} exp(sim_ab/tau),  sim = Z Z^T,  Z = row-normalized
  concat(e_i, e_j).

Sharding: row-parallel over the 8192 rows of the similarity matrix. Every
core receives the full embedding matrix rotated by -c*1024 rows so that its
1024 rows are always local rows 0..1023 (identical SPMD program on all
cores). Each core computes row sums of exp(sim/tau) for its rows against
all 8192 columns (fused exp+accumulate on the scalar engine), plus the
per-row self-similarity (z2) and positive-pair dot (pos). The host performs
the final gather: den = rowsum - exp(z2/tau), loss = mean(log den - pos/tau).

Engine budget per core: ACT does 8.4M exp (the bottleneck, ~66us); PE does
the 1024x8192x128 fp32r matmul plus 64 transposes; DVE does normalization
(squares, row reductions, a magic-constant+Newton rsqrt -- keeps ACT free
of Sqrt/Ln table loads), scaling, and the PSUM->SBUF float32r copies.

Note: tensor_tensor_reduce (custom DVE ISA op) hangs this runtime -- all
row reductions use tensor_tensor + tensor_reduce or ACT accum_out instead.
"""

import numpy as np

B = 4096
TB = 2 * B      # 8192 rows of reps
D = 128
TAU = 0.5
N_CORES = 8
R = TB // N_CORES   # 1024 rows per core
MT = R // 128       # 8 row-tiles owned per core
NT = TB // 128      # 64 row-tiles total
G = 4               # column supergroups
GT = NT // G        # 16 row-tiles per supergroup
GC = TB // G        # 2048 columns per supergroup

MAGIC = 0x5F3759DF  # fast inverse-sqrt initial guess

_CACHE = {}


def _build():
    import concourse.tile as tile
    from concourse import bacc, mybir

    f32 = mybir.dt.float32
    f32r = mybir.dt.float32r
    i32 = mybir.dt.int32
    Exp = mybir.ActivationFunctionType.Exp
    OpAdd = mybir.AluOpType.add
    OpMult = mybir.AluOpType.mult
    OpShr = mybir.AluOpType.arith_shift_right
    OpXor = mybir.AluOpType.bitwise_xor
    AxisX = mybir.AxisListType.X

    nc = bacc.Bacc(
        "TRN2", target_bir_lowering=False, debug=False, num_devices=N_CORES
    )
    e_ap = nc.dram_tensor("e", [TB, D], f32, kind="ExternalInput").ap()
    ident_ap = nc.dram_tensor("ident", [128, 128], f32, kind="ExternalInput").ap()
    rs_ap = nc.dram_tensor("rs", [128, MT], f32, kind="ExternalOutput").ap()
    pos_ap = nc.dram_tensor("pos", [128, MT], f32, kind="ExternalOutput").ap()
    z2_ap = nc.dram_tensor("z2", [128, MT], f32, kind="ExternalOutput").ap()

    with tile.TileContext(nc) as tc:
        with (
            tc.tile_pool(name="xp", bufs=1) as xp,
            tc.tile_pool(name="ztp", bufs=1) as ztp,
            tc.tile_pool(name="small", bufs=1) as sp,
            tc.tile_pool(name="sq", bufs=2) as sqp,
            tc.tile_pool(name="ps", bufs=2, space="PSUM") as pp,
        ):
            ident = sp.tile([128, 128], f32, tag="ident")
            nc.scalar.dma_start(ident[:], ident_ap[:])
            # Dummy exp right after the ident load: pulls the one ACT
            # table load off the critical path (overlaps input DMA).
            dummy = sp.tile([128, 1], f32, tag="dummy")
            nc.scalar.activation(dummy[:], ident[:, 0:1], Exp)

            # Raw rows: one [128, 16*128] tile per supergroup, loaded with a
            # single strided DMA (HWDGE queues alternate between groups).
            # Scaled in place to Z after normalization.
            dma_engines = [nc.sync, nc.scalar]
            xgs = []
            for g in range(G):
                xg = xp.tile([128, GC], f32, tag=f"xg{g}", name=f"xg{g}")
                if g < 2:
                    # Latency-critical early groups: split across both HWDGE
                    # queues so group 0 lands in half the time.
                    half = GC // 2
                    for h in range(2):
                        rows = slice(g * GC + h * half, g * GC + (h + 1) * half)
                        src = e_ap[rows, :].rearrange("(j p) d -> p j d", p=128)
                        dst = xg[:, h * half : (h + 1) * half].rearrange(
                            "p (j d) -> p j d", d=128
                        )
                        dma_engines[h].dma_start(dst, src)
                else:
                    src = e_ap[g * GC : (g + 1) * GC, :].rearrange(
                        "(j p) d -> p j d", p=128
                    )
                    dst = xg[:].rearrange("p (j d) -> p j d", d=128)
                    dma_engines[g % 2].dma_start(dst, src)
                xgs.append(xg)

            def xtile(t):
                g, j = divmod(t, GT)
                return xgs[g][:, j * 128 : (j + 1) * 128]

            s2 = sp.tile([128, NT], f32, tag="s2")
            inv = sp.tile([128, NT], f32, tag="inv")
            nrt = sp.tile([128, NT], f32, tag="nrt")
            parts = sp.tile([128, MT * (G - 1)], f32, tag="parts")
            partsb = sp.tile([128, MT], f32, tag="partsb")
            rs2 = sp.tile([128, MT], f32, tag="rs2")
            rs_t = sp.tile([128, MT], f32, tag="rs")
            pos_t = sp.tile([128, MT], f32, tag="pos")
            z2_t = sp.tile([128, MT], f32, tag="z2")
            inv2 = sp.tile([128, MT], f32, tag="inv2")

            # Transposed normalized rows, one [128(d), 2048(rows)] tile per
            # supergroup, rounded to float32r for the PE's single-pass fp32
            # matmul mode (the DVE copy out of PSUM performs the rounding).
            # ZT group 0 also holds this core's own 1024 rows.
            zts = [
                ztp.tile([128, GC], f32r, tag=f"zt{g}", name=f"zt{g}")
                for g in range(G)
            ]

            def rsqrt(cols):
                """inv[:, cols] = 1/sqrt(s2[:, cols]) via magic guess + two
                Newton steps, entirely on DVE (no ACT table switches).
                MAGIC - x == ~x + (MAGIC+1) avoids a reverse-subtract op."""
                s2i = s2[:, cols].bitcast(i32)
                invi = inv[:, cols].bitcast(i32)
                nc.vector.tensor_scalar(
                    out=invi, in0=s2i, scalar1=1, scalar2=-1,
                    op0=OpShr, op1=OpXor,
                )
                nc.vector.tensor_scalar(
                    out=invi, in0=invi, scalar1=MAGIC + 1, scalar2=None, op0=OpAdd
                )
                for _ in range(2):
                    nr = nrt[:, cols]
                    nc.vector.tensor_tensor(nr, inv[:, cols], inv[:, cols], OpMult)
                    nc.vector.tensor_tensor(nr, nr, s2[:, cols], OpMult)
                    nc.vector.tensor_scalar(
                        out=nr, in0=nr, scalar1=-0.5, scalar2=1.5,
                        op0=OpMult, op1=OpAdd,
                    )
                    nc.vector.tensor_tensor(inv[:, cols], inv[:, cols], nr, OpMult)

            def prep_group(g):
                gcols = slice(g * GT, (g + 1) * GT)
                # squares -> batched row-reduce -> s2 for the group's tiles
                sq = sqp.tile([128, GC], f32, tag="sq", name=f"sq{g}")
                for j in range(GT):
                    t = g * GT + j
                    nc.vector.tensor_tensor(
                        sq[:, j * 128 : (j + 1) * 128], xtile(t), xtile(t), OpMult
                    )
                sq3 = sq[:].rearrange("p (j d) -> p j d", d=128)
                nc.vector.tensor_reduce(s2[:, gcols], sq3, axis=AxisX, op=OpAdd)
                rsqrt(gcols)
                for j in range(GT):
                    t = g * GT + j
                    nc.vector.tensor_scalar_mul(xtile(t), xtile(t), inv[:, t : t + 1])
                tp = pp.tile([128, GC], f32, tag="ps", name=f"tp{g}")
                for j in range(GT):
                    t = g * GT + j
                    nc.tensor.transpose(tp[:, j * 128 : (j + 1) * 128], xtile(t), ident[:])
                # Chunked copy-out (DVE; DMA cannot read PSUM) so the PSUM
                # slot frees progressively. Converts fp32 -> float32r.
                for j in range(4):
                    cols = slice(j * 512, (j + 1) * 512)
                    nc.vector.tensor_copy(zts[g][:, cols], tp[:, cols])

            def mm_group(g, m):
                mm = pp.tile([128, GC], f32, tag="ps", name=f"mm{g}_{m}")
                lhsT = zts[0][:, m * 128 : (m + 1) * 128]
                for j in range(4):
                    cols = slice(j * 512, (j + 1) * 512)
                    nc.tensor.matmul(mm[:, cols], lhsT, zts[g][:, cols])
                # exp in place in PSUM (discarded); accum_out is the row sum.
                if g < G - 1:
                    acc = parts[:, m * (G - 1) + g : m * (G - 1) + g + 1]
                else:
                    acc = partsb[:, m : m + 1]
                nc.scalar.activation(
                    mm[:], mm[:], Exp, scale=1.0 / TAU, accum_out=acc,
                )

            prep_group(0)
            for g in range(G):
                if g == 1:
                    # z2 (self-similarity of my rows) = s2 * inv^2, from
                    # group-0 values.
                    nc.vector.tensor_tensor(
                        inv2[:], inv[:, :MT], inv[:, :MT], OpMult
                    )
                    nc.vector.tensor_tensor(z2_t[:], inv2[:], s2[:, :MT], OpMult)
                if g == 3:
                    # Positive pairs: my local row l pairs with local row
                    # l + 4096 = tile m + 32 (holds for both halves under
                    # the rotation). Tiles 32..39 are scaled by prep_group(2).
                    psq = sqp.tile([128, MT * 128], f32, tag="sq", name="psq")
                    for m in range(MT):
                        nc.vector.tensor_tensor(
                            psq[:, m * 128 : (m + 1) * 128],
                            xtile(m),
                            xtile(m + NT // 2),
                            OpMult,
                        )
                    psq3 = psq[:].rearrange("p (m d) -> p m d", d=128)
                    nc.vector.tensor_reduce(pos_t[:], psq3, axis=AxisX, op=OpAdd)
                if g == 3:
                    # Hoist the g<3 reduction into the last group's window;
                    # only a [128,8] add remains after the final exp.
                    parts3 = parts[:].rearrange("p (m g) -> p m g", g=G - 1)
                    nc.vector.tensor_reduce(rs2[:], parts3, axis=AxisX, op=OpAdd)
                for m in range(MT):
                    if g < G - 1 and m == 3:
                        prep_group(g + 1)
                    mm_group(g, m)

            # rs = (hoisted sum of g<3 parts) + g=3 parts.
            nc.vector.tensor_tensor(rs_t[:], rs2[:], partsb[:], OpAdd)

            nc.sync.dma_start(rs_ap[:], rs_t[:])
            nc.sync.dma_start(pos_ap[:], pos_t[:])
            nc.sync.dma_start(z2_ap[:], z2_t[:])

    nc.compile()
    return nc


def _get_nc():
    if "nc" not in _CACHE:
        _CACHE["nc"] = _build()
    return _CACHE["nc"]


def kernel(e_i: np.ndarray, e_j: np.ndarray, _trace: bool = False):
    from concourse.bass_utils import run_bass_kernel_spmd

    nc = _get_nc()
    e = np.concatenate(
        [np.asarray(e_i, np.float32), np.asarray(e_j, np.float32)], axis=0
    )
    ident = np.eye(128, dtype=np.float32)
    in_maps = [
        {"e": np.ascontiguousarray(np.roll(e, -c * R, axis=0)), "ident": ident}
        for c in range(N_CORES)
    ]
    res = run_bass_kernel_spmd(nc, in_maps, list(range(N_CORES)), trace=_trace)
    _CACHE["last_exec_time_ns"] = res.exec_time_ns
    _CACHE["last_res"] = res

    rs = np.empty(TB, np.float64)
    z2 = np.empty(TB, np.float64)
    pos = np.empty(TB, np.float64)
    for c in range(N_CORES):
        o = res.results[c]
        rows = slice(c * R, (c + 1) * R)
        # out[p, m] is local row m*128+p -> transpose to row-major order.
        rs[rows] = o["rs"].astype(np.float64).T.reshape(-1)
        z2[rows] = o["z2"].astype(np.float64).T.reshape(-1)
        pos[rows] = o["pos"].astype(np.float64).T.reshape(-1)

    den = rs - np.exp(z2 / TAU)
    loss = np.mean(np.log(den) - pos / TAU)
    return np.float32(loss)



# revision 3
# speedup vs baseline: 1.0706x; 1.0706x over previous
"""NT-Xent loss Trainium2 kernel (8-core SPMD, Bass/Tile).

Math: loss = mean_a [ log(den_a) - pos_a/tau ],
  den_a = sum_{b != a} exp(sim_ab/tau),  sim = Z Z^T,  Z = row-normalized
  concat(e_i, e_j).

Sharding: row-parallel over the 8192 rows of the similarity matrix. Every
core receives the full embedding matrix (bf16) rotated by -c*1024 rows so
its 1024 rows are local rows 0..1023 (identical SPMD program on all cores).
Each core computes row sums of exp(sim/tau) for its rows against all 8192
columns (fused exp+accumulate on the scalar engine), plus per-row
self-similarity (z2) and positive-pair dot (pos). Host gathers:
den = rowsum - exp(z2/tau), loss = mean(log den - pos/tau).

Column supergroups are variable-sized [512, 1536, 2048, 2048, 2048] so the
first exp only waits on a 512-column prep chain (ramp ~5us instead of 22us).
All matmul operands are bf16 (PE ~3x faster than fp32r; tolerance is 2e-2
and bf16 keeps the loss within ~1e-4).

Engine budget per core: ACT does 8.4M exp (bottleneck, ~66us); PE does the
bf16 matmuls plus 64 transposes; DVE does normalization (squares, row
reductions, magic-constant+Newton rsqrt -- keeps ACT free of Sqrt/Ln table
loads), scaling, and the PSUM->SBUF copies.
"""

import numpy as np

B = 4096
TB = 2 * B      # 8192 rows of reps
D = 128
TAU = 0.5
N_CORES = 8
R = TB // N_CORES   # 1024 rows per core
MT = R // 128       # 8 row-tiles owned per core
NT = TB // 128      # 64 row-tiles total

# Variable column supergroups (in col units); first group small for fast ramp.
GSIZES = [512, 1536, 2048, 2048, 2048]
GOFF = [0, 512, 2048, 4096, 6144]
G = len(GSIZES)
NPARTS = G - 1      # accum parts per row-tile for g<G-1

MAGIC = 0x5F3759DF  # fast inverse-sqrt initial guess

_CACHE = {}


def _build():
    import concourse.tile as tile
    from concourse import bacc, mybir

    f32 = mybir.dt.float32
    bf16 = mybir.dt.bfloat16
    i32 = mybir.dt.int32
    Exp = mybir.ActivationFunctionType.Exp
    OpAdd = mybir.AluOpType.add
    OpMult = mybir.AluOpType.mult
    OpShr = mybir.AluOpType.arith_shift_right
    OpXor = mybir.AluOpType.bitwise_xor
    AxisX = mybir.AxisListType.X

    nc = bacc.Bacc(
        "TRN2", target_bir_lowering=False, debug=False, num_devices=N_CORES
    )
    e_ap = nc.dram_tensor("e", [TB, D], bf16, kind="ExternalInput").ap()
    ident_ap = nc.dram_tensor("ident", [128, 128], bf16, kind="ExternalInput").ap()
    rs_ap = nc.dram_tensor("rs", [128, MT], f32, kind="ExternalOutput").ap()
    pos_ap = nc.dram_tensor("pos", [128, MT], f32, kind="ExternalOutput").ap()
    z2_ap = nc.dram_tensor("z2", [128, MT], f32, kind="ExternalOutput").ap()

    with tile.TileContext(nc) as tc:
        with (
            tc.tile_pool(name="xp", bufs=1) as xp,
            tc.tile_pool(name="ztp", bufs=1) as ztp,
            tc.tile_pool(name="small", bufs=1) as sp,
            tc.tile_pool(name="sq", bufs=2) as sqp,
            tc.tile_pool(name="ps", bufs=2, space="PSUM") as pp,
        ):
            ident = sp.tile([128, 128], bf16, tag="ident")
            nc.scalar.dma_start(ident[:], ident_ap[:])
            # Dummy exp right after the ident load: pulls the one ACT
            # table load off the critical path (overlaps input DMA).
            dummy = sp.tile([128, 1], f32, tag="dummy")
            nc.scalar.activation(dummy[:], ident[:, 0:1], Exp)

            # Raw rows: one [128, GCg] tile per supergroup, strided DMA.
            # Group 0 split across both HWDGE queues for minimum latency.
            dma_engines = [nc.sync, nc.scalar]
            xgs = []
            for g in range(G):
                gc = GSIZES[g]
                xg = xp.tile([128, gc], bf16, tag=f"xg{g}", name=f"xg{g}")
                if g < 2:
                    half = gc // 2
                    for h in range(2):
                        rows = slice(GOFF[g] + h * half, GOFF[g] + (h + 1) * half)
                        src = e_ap[rows, :].rearrange("(j p) d -> p j d", p=128)
                        dst = xg[:, h * half : (h + 1) * half].rearrange(
                            "p (j d) -> p j d", d=128
                        )
                        dma_engines[h].dma_start(dst, src)
                else:
                    src = e_ap[GOFF[g] : GOFF[g] + gc, :].rearrange(
                        "(j p) d -> p j d", p=128
                    )
                    dst = xg[:].rearrange("p (j d) -> p j d", d=128)
                    dma_engines[g % 2].dma_start(dst, src)
                xgs.append(xg)

            def xtile(t):
                # global tile index t -> (group, slice)
                for g in range(G):
                    t0 = GOFF[g] // 128
                    nt = GSIZES[g] // 128
                    if t < t0 + nt:
                        j = t - t0
                        return xgs[g][:, j * 128 : (j + 1) * 128]
                raise IndexError(t)

            s2 = sp.tile([128, NT], f32, tag="s2")
            inv = sp.tile([128, NT], f32, tag="inv")
            nrt = sp.tile([128, NT], f32, tag="nrt")
            parts = sp.tile([128, MT * NPARTS], f32, tag="parts")
            partsb = sp.tile([128, MT], f32, tag="partsb")
            rs2 = sp.tile([128, MT], f32, tag="rs2")
            rs_t = sp.tile([128, MT], f32, tag="rs")
            pos_t = sp.tile([128, MT], f32, tag="pos")
            z2_t = sp.tile([128, MT], f32, tag="z2")
            inv2 = sp.tile([128, MT], f32, tag="inv2")

            # Transposed normalized rows (bf16), one tile per supergroup.
            zts = [
                ztp.tile([128, GSIZES[g]], bf16, tag=f"zt{g}", name=f"zt{g}")
                for g in range(G)
            ]

            def rsqrt(cols):
                """inv[:, cols] = 1/sqrt(s2[:, cols]) via magic guess + two
                Newton steps, entirely on DVE (no ACT table switches)."""
                s2i = s2[:, cols].bitcast(i32)
                invi = inv[:, cols].bitcast(i32)
                nc.vector.tensor_scalar(
                    out=invi, in0=s2i, scalar1=1, scalar2=-1,
                    op0=OpShr, op1=OpXor,
                )
                nc.vector.tensor_scalar(
                    out=invi, in0=invi, scalar1=MAGIC + 1, scalar2=None, op0=OpAdd
                )
                for _ in range(2):
                    nr = nrt[:, cols]
                    nc.vector.tensor_tensor(nr, inv[:, cols], inv[:, cols], OpMult)
                    nc.vector.tensor_tensor(nr, nr, s2[:, cols], OpMult)
                    nc.vector.tensor_scalar(
                        out=nr, in0=nr, scalar1=-0.5, scalar2=1.5,
                        op0=OpMult, op1=OpAdd,
                    )
                    nc.vector.tensor_tensor(inv[:, cols], inv[:, cols], nr, OpMult)

            def prep_group(g):
                gc = GSIZES[g]
                t0 = GOFF[g] // 128
                nt = gc // 128
                gcols = slice(t0, t0 + nt)
                sq = sqp.tile([128, gc], f32, tag="sq", name=f"sq{g}")
                for j in range(nt):
                    nc.vector.tensor_tensor(
                        sq[:, j * 128 : (j + 1) * 128],
                        xtile(t0 + j), xtile(t0 + j), OpMult,
                    )
                sq3 = sq[:].rearrange("p (j d) -> p j d", d=128)
                nc.vector.tensor_reduce(s2[:, gcols], sq3, axis=AxisX, op=OpAdd)
                rsqrt(gcols)
                for j in range(nt):
                    t = t0 + j
                    nc.vector.tensor_scalar_mul(xtile(t), xtile(t), inv[:, t : t + 1])
                # Transpose in <=1024-col chunks so the borrowed PSUM slot is
                # held briefly (shorter ACT stalls at group boundaries).
                done = 0
                while done < nt:
                    cn = min(8, nt - done)
                    tp = pp.tile([128, 2048], bf16, tag="ps", name=f"tp{g}_{done}")
                    for j in range(cn):
                        t = t0 + done + j
                        nc.tensor.transpose(
                            tp[:, j * 128 : (j + 1) * 128], xtile(t), ident[:]
                        )
                    nc.vector.tensor_copy(
                        zts[g][:, done * 128 : (done + cn) * 128],
                        tp[:, : cn * 128],
                    )
                    done += cn

            def zttile(t):
                for g in range(G):
                    t0 = GOFF[g] // 128
                    nt = GSIZES[g] // 128
                    if t < t0 + nt:
                        j = t - t0
                        return zts[g][:, j * 128 : (j + 1) * 128]
                raise IndexError(t)

            def mm_group(g, m):
                gc = GSIZES[g]
                mm = pp.tile([128, 2048], f32, tag="ps", name=f"mm{g}_{m}")
                lhsT = zttile(m)
                for j in range(0, gc, 512):
                    nc.tensor.matmul(
                        mm[:, j : j + 512], lhsT, zts[g][:, j : j + 512]
                    )
                if g < G - 1:
                    acc = parts[:, m * NPARTS + g : m * NPARTS + g + 1]
                else:
                    acc = partsb[:, m : m + 1]
                nc.scalar.activation(
                    mm[:, :gc], mm[:, :gc], Exp, scale=1.0 / TAU, accum_out=acc,
                )

            prep_group(0)
            for g in range(G):
                if g == 1:
                    # z2 (self-similarity of my rows) = s2 * inv^2, from
                    # group-0 values (tiles 0..3) + group-1 (tiles 4..7).
                    nc.vector.tensor_tensor(
                        inv2[:], inv[:, :MT], inv[:, :MT], OpMult
                    )
                    nc.vector.tensor_tensor(z2_t[:], inv2[:], s2[:, :MT], OpMult)
                if g == 4:
                    # Positive pairs: local row l pairs with local row l+4096
                    # = tile m+32 (scaled by prep_group(3), done during g==2).
                    psq = sqp.tile([128, MT * 128], f32, tag="sq", name="psq")
                    for m in range(MT):
                        nc.vector.tensor_tensor(
                            psq[:, m * 128 : (m + 1) * 128],
                            xtile(m),
                            xtile(m + NT // 2),
                            OpMult,
                        )
                    psq3 = psq[:].rearrange("p (m d) -> p m d", d=128)
                    nc.vector.tensor_reduce(pos_t[:], psq3, axis=AxisX, op=OpAdd)
                    # Hoist the g<4 reduction into the last group's window.
                    parts3 = parts[:].rearrange("p (m g) -> p m g", g=NPARTS)
                    nc.vector.tensor_reduce(rs2[:], parts3, axis=AxisX, op=OpAdd)
                for m in range(MT):
                    if g < G - 1 and m == 1:
                        prep_group(g + 1)
                    mm_group(g, m)

            # rs = (hoisted sum of g<4 parts) + g=4 parts.
            nc.vector.tensor_tensor(rs_t[:], rs2[:], partsb[:], OpAdd)

            nc.sync.dma_start(rs_ap[:], rs_t[:])
            nc.sync.dma_start(pos_ap[:], pos_t[:])
            nc.sync.dma_start(z2_ap[:], z2_t[:])

    nc.compile()
    return nc


def _get_nc():
    if "nc" not in _CACHE:
        _CACHE["nc"] = _build()
    return _CACHE["nc"]


def kernel(e_i: np.ndarray, e_j: np.ndarray, _trace: bool = False):
    import ml_dtypes
    from concourse.bass_utils import run_bass_kernel_spmd

    bf16 = ml_dtypes.bfloat16
    nc = _get_nc()
    e = np.concatenate(
        [np.asarray(e_i, np.float32), np.asarray(e_j, np.float32)], axis=0
    ).astype(bf16)
    ident = np.eye(128, dtype=bf16)
    in_maps = [
        {"e": np.ascontiguousarray(np.roll(e, -c * R, axis=0)), "ident": ident}
        for c in range(N_CORES)
    ]
    res = run_bass_kernel_spmd(nc, in_maps, list(range(N_CORES)), trace=_trace)
    _CACHE["last_exec_time_ns"] = res.exec_time_ns
    _CACHE["last_res"] = res

    rs = np.empty(TB, np.float64)
    z2 = np.empty(TB, np.float64)
    pos = np.empty(TB, np.float64)
    for c in range(N_CORES):
        o = res.results[c]
        rows = slice(c * R, (c + 1) * R)
        # out[p, m] is local row m*128+p -> transpose to row-major order.
        rs[rows] = o["rs"].astype(np.float64).T.reshape(-1)
        z2[rows] = o["z2"].astype(np.float64).T.reshape(-1)
        pos[rows] = o["pos"].astype(np.float64).T.reshape(-1)

    den = rs - np.exp(z2 / TAU)
    loss = np.mean(np.log(den) - pos / TAU)
    return np.float32(loss)


# revision 8
# speedup vs baseline: 1.0882x; 1.0164x over previous
"""NT-Xent loss Trainium2 kernel (8-core SPMD, Bass/Tile).

Math: loss = mean_a [ log(den_a) - pos_a/tau ],
  den_a = sum_{b != a} exp(sim_ab/tau),  sim = Z Z^T,  Z = row-normalized
  concat(e_i, e_j).

Sharding: row-parallel over the 8192 rows of the similarity matrix. Every
core receives the full embedding matrix (bf16) rotated by -c*1024 rows so
its 1024 rows are local rows 0..1023 (identical SPMD program on all cores).
The host ALSO pre-transposes to [128(p), 64(j), 128(d)] so each partition's
DMA is one long contiguous run (256B-packet strided loads were ~100GB/s and
dominated the ramp). Each core computes row sums of exp(sim/tau) for its
rows against all 8192 columns (fused exp+accumulate on the scalar engine),
plus per-row self-similarity (z2) and positive-pair dot (pos). Host
gathers: den = rowsum - exp(z2/tau), loss = mean(log den - pos/tau).

Column supergroups are variable-sized [512, 1536, 2048, 2048, 2048] so the
first exp only waits on a 512-column prep chain. All matmul operands are
bf16 (PE ~3x faster than fp32r; tolerance is 2e-2, bf16 keeps the loss
within ~1e-4). Prep transposes for group g+1 are issued in PAIRS of PSUM
allocations between mm_groups so the 2-slot PSUM rotation keeps consecutive
mm tiles in opposite slots (parity), avoiding ACT stalls.
"""

import numpy as np

B = 4096
TB = 2 * B      # 8192 rows of reps
D = 128
TAU = 0.5
N_CORES = 8
R = TB // N_CORES   # 1024 rows per core
MT = R // 128       # 8 row-tiles owned per core
NT = TB // 128      # 64 row-tiles total

# Variable column supergroups (in col units); first group small for fast ramp.
GSIZES = [512, 1536, 2048, 2048, 2048]
GOFF = [0, 512, 2048, 4096, 6144]
G = len(GSIZES)
NPARTS = G - 1      # accum parts per row-tile for g<G-1

MAGIC = 0x5F3759DF  # fast inverse-sqrt initial guess

_CACHE = {}


def _build():
    import concourse.tile as tile
    from concourse import bacc, mybir

    f32 = mybir.dt.float32
    bf16 = mybir.dt.bfloat16
    i32 = mybir.dt.int32
    Exp = mybir.ActivationFunctionType.Exp
    OpAdd = mybir.AluOpType.add
    OpMult = mybir.AluOpType.mult
    OpShr = mybir.AluOpType.arith_shift_right
    OpXor = mybir.AluOpType.bitwise_xor
    AxisX = mybir.AxisListType.X

    nc = bacc.Bacc(
        "TRN2", target_bir_lowering=False, debug=False, num_devices=N_CORES
    )
    # e is pre-rotated AND pre-transposed on the host: [128(p), NT(j)*128(d)],
    # so row j*128+p of the rotated matrix is e[p, j*128:(j+1)*128].
    e_ap = nc.dram_tensor("e", [128, NT * D], bf16, kind="ExternalInput").ap()
    ident_ap = nc.dram_tensor("ident", [128, 128], bf16, kind="ExternalInput").ap()
    rs_ap = nc.dram_tensor("rs", [128, MT], f32, kind="ExternalOutput").ap()
    pos_ap = nc.dram_tensor("pos", [128, MT], f32, kind="ExternalOutput").ap()
    z2_ap = nc.dram_tensor("z2", [128, MT], f32, kind="ExternalOutput").ap()

    with tile.TileContext(nc) as tc:
        with (
            tc.tile_pool(name="xp", bufs=1) as xp,
            tc.tile_pool(name="ztp", bufs=1) as ztp,
            tc.tile_pool(name="small", bufs=1) as sp,
            tc.tile_pool(name="sq", bufs=2) as sqp,
            tc.tile_pool(name="ps", bufs=2, space="PSUM") as pp,
        ):
            ident = sp.tile([128, 128], bf16, tag="ident")
            nc.sync.dma_start(ident[:], ident_ap[:])
            # Dummy exp right after the ident load: pulls the one ACT
            # table load off the critical path (overlaps input DMA).
            dummy = sp.tile([128, 1], f32, tag="dummy")
            nc.scalar.activation(dummy[:], ident[:, 0:1], Exp)

            # Raw rows: one [128, GCg] tile per supergroup. Contiguous free
            # dim per partition -> long DMA lines at full HBM bandwidth.
            # ALL input DMA on the sync queue IN GROUP ORDER: FIFO guarantees
            # g0 lands first, g1 next, ... (a split queue plan let g1 land
            # behind g3 and stalled the DVE pipeline).
            xgs = []
            for g in range(G):
                gc = GSIZES[g]
                xg = xp.tile([128, gc], bf16, tag=f"xg{g}", name=f"xg{g}")
                cs = slice(GOFF[g], GOFF[g] + gc)
                nc.sync.dma_start(xg[:], e_ap[:, cs])
                xgs.append(xg)

            def xtile(t):
                # global tile index t -> (group, slice)
                for g in range(G):
                    t0 = GOFF[g] // 128
                    nt = GSIZES[g] // 128
                    if t < t0 + nt:
                        j = t - t0
                        return xgs[g][:, j * 128 : (j + 1) * 128]
                raise IndexError(t)

            s2 = sp.tile([128, NT], f32, tag="s2")
            inv = sp.tile([128, NT], f32, tag="inv")
            nrt = sp.tile([128, NT], f32, tag="nrt")
            parts = sp.tile([128, MT * NPARTS], f32, tag="parts")
            partsb = sp.tile([128, MT], f32, tag="partsb")
            rs2 = sp.tile([128, MT], f32, tag="rs2")
            rs_t = sp.tile([128, MT], f32, tag="rs")
            pos_t = sp.tile([128, MT], f32, tag="pos")
            z2_t = sp.tile([128, MT], f32, tag="z2")
            inv2 = sp.tile([128, MT], f32, tag="inv2")

            # Transposed normalized rows (bf16), one tile per supergroup.
            zts = [
                ztp.tile([128, GSIZES[g]], bf16, tag=f"zt{g}", name=f"zt{g}")
                for g in range(G)
            ]

            def zttile(t):
                for g in range(G):
                    t0 = GOFF[g] // 128
                    nt = GSIZES[g] // 128
                    if t < t0 + nt:
                        j = t - t0
                        return zts[g][:, j * 128 : (j + 1) * 128]
                raise IndexError(t)

            def rsqrt(cols):
                """inv[:, cols] = 1/sqrt(s2[:, cols]) via magic guess + one
                Newton step, entirely on DVE (no ACT table switches).
                One step leaves ~0.1% relative error; systematic row-scale
                errors cancel in the mean over 8192 rows (tolerance 2e-2)."""
                s2i = s2[:, cols].bitcast(i32)
                invi = inv[:, cols].bitcast(i32)
                nc.vector.tensor_scalar(
                    out=invi, in0=s2i, scalar1=1, scalar2=-1,
                    op0=OpShr, op1=OpXor,
                )
                nc.vector.tensor_scalar(
                    out=invi, in0=invi, scalar1=MAGIC + 1, scalar2=None, op0=OpAdd
                )
                for _ in range(1):
                    nr = nrt[:, cols]
                    nc.vector.tensor_tensor(nr, inv[:, cols], inv[:, cols], OpMult)
                    nc.vector.tensor_tensor(nr, nr, s2[:, cols], OpMult)
                    nc.vector.tensor_scalar(
                        out=nr, in0=nr, scalar1=-0.5, scalar2=1.5,
                        op0=OpMult, op1=OpAdd,
                    )
                    nc.vector.tensor_tensor(inv[:, cols], inv[:, cols], nr, OpMult)

            def prep_dve(g):
                """squares -> row sumsq -> rsqrt -> scale rows (all DVE)."""
                gc = GSIZES[g]
                t0 = GOFF[g] // 128
                nt = gc // 128
                gcols = slice(t0, t0 + nt)
                sq = sqp.tile([128, gc], f32, tag="sq", name=f"sq{g}")
                for j in range(nt):
                    nc.vector.tensor_tensor(
                        sq[:, j * 128 : (j + 1) * 128],
                        xtile(t0 + j), xtile(t0 + j), OpMult,
                    )
                sq3 = sq[:].rearrange("p (j d) -> p j d", d=128)
                nc.vector.tensor_reduce(s2[:, gcols], sq3, axis=AxisX, op=OpAdd)
                rsqrt(gcols)
                for j in range(nt):
                    t = t0 + j
                    nc.vector.tensor_scalar_mul(xtile(t), xtile(t), inv[:, t : t + 1])

            def prep_tp(g, j0, cn, ncopy=1):
                """Transpose cn tiles (starting at local j0) of group g into
                zts[g] via one PSUM allocation; copy out in ncopy chunks so
                the first chunk is usable before the last transpose."""
                t0 = GOFF[g] // 128
                tp = pp.tile([128, 2048], bf16, tag="ps", name=f"tp{g}_{j0}")
                for j in range(cn):
                    nc.tensor.transpose(
                        tp[:, j * 128 : (j + 1) * 128], xtile(t0 + j0 + j), ident[:]
                    )
                cc = cn // ncopy
                for k in range(0, cn, cc):
                    nc.vector.tensor_copy(
                        zts[g][:, (j0 + k) * 128 : (j0 + k + cc) * 128],
                        tp[:, k * 128 : (k + cc) * 128],
                    )

            def mm_group(g, m):
                gc = GSIZES[g]
                mm = pp.tile([128, 2048], f32, tag="ps", name=f"mm{g}_{m}")
                lhsT = zttile(m)
                for j in range(0, gc, 512):
                    nc.tensor.matmul(
                        mm[:, j : j + 512], lhsT, zts[g][:, j : j + 512]
                    )
                if g < G - 1:
                    acc = parts[:, m * NPARTS + g : m * NPARTS + g + 1]
                else:
                    acc = partsb[:, m : m + 1]
                nc.scalar.activation(
                    mm[:, :gc], mm[:, :gc], Exp, scale=1.0 / TAU, accum_out=acc,
                )

            # Sim-time scheduling anchors (ms): hold next-group prep work out
            # of the current group's critical window so the list scheduler
            # cannot place not-yet-ready ops ahead of ready pipeline ops on
            # the in-order engines. These are hints only (no real waits).
            GSTART = [0.0045, 0.010, 0.0225, 0.039, 0.0555]  # sim start of grp
            # Group 0 prep (ramp-critical): dve chain + one tp chunk.
            prep_dve(0)
            prep_tp(0, 0, 4, ncopy=2)

            for g in range(G):
                nt_next = GSIZES[g + 1] // 128 if g < G - 1 else 0
                if g == 1:
                    # z2 (self-similarity of my rows) = s2 * inv^2 from
                    # group-0/1 values (tiles 0..7).
                    with tc.tile_wait_until(GSTART[1]):
                        nc.vector.tensor_tensor(
                            inv2[:], inv[:, :MT], inv[:, :MT], OpMult
                        )
                        nc.vector.tensor_tensor(
                            z2_t[:], inv2[:], s2[:, :MT], OpMult
                        )
                if g == 4:
                    # Positive pairs: local row l pairs with local row l+4096
                    # = tile m+32 (scaled by prep_dve(3), done during g==2).
                    with tc.tile_wait_until(GSTART[4]):
                        psq = sqp.tile([128, MT * 128], f32, tag="sq", name="psq")
                        for m in range(MT):
                            nc.vector.tensor_tensor(
                                psq[:, m * 128 : (m + 1) * 128],
                                xtile(m),
                                xtile(m + NT // 2),
                                OpMult,
                            )
                        psq3 = psq[:].rearrange("p (m d) -> p m d", d=128)
                        nc.vector.tensor_reduce(pos_t[:], psq3, axis=AxisX, op=OpAdd)
                        # Hoist the g<4 reduction into the last group's window.
                        parts3 = parts[:].rearrange("p (m g) -> p m g", g=NPARTS)
                        nc.vector.tensor_reduce(rs2[:], parts3, axis=AxisX, op=OpAdd)
                for m in range(MT):
                    if g < G - 1 and m == 0:
                        with tc.tile_wait_until(GSTART[g]):
                            prep_dve(g + 1)
                    # Transposes for next group in PAIRS of PSUM allocations
                    # (keeps mm slot parity) late in this group's window.
                    if g < G - 1 and m == 3:
                        h = nt_next // 2
                        mid = GSTART[g] + 0.45 * (GSTART[g + 1] - GSTART[g])
                        mid2 = GSTART[g] + 0.7 * (GSTART[g + 1] - GSTART[g])
                        with tc.tile_wait_until(mid):
                            prep_tp(g + 1, 0, h)
                        with tc.tile_wait_until(mid2):
                            prep_tp(g + 1, h, nt_next - h)
                    mm_group(g, m)

            # rs = (hoisted sum of g<4 parts) + g=4 parts.
            nc.vector.tensor_tensor(rs_t[:], rs2[:], partsb[:], OpAdd)

            nc.sync.dma_start(rs_ap[:], rs_t[:])
            nc.sync.dma_start(pos_ap[:], pos_t[:])
            nc.sync.dma_start(z2_ap[:], z2_t[:])

    nc.compile()
    return nc


def _get_nc():
    if "nc" not in _CACHE:
        _CACHE["nc"] = _build()
    return _CACHE["nc"]


def kernel(e_i: np.ndarray, e_j: np.ndarray, _trace: bool = False):
    import ml_dtypes
    from concourse.bass_utils import run_bass_kernel_spmd

    bf16 = ml_dtypes.bfloat16
    nc = _get_nc()
    e = np.concatenate(
        [np.asarray(e_i, np.float32), np.asarray(e_j, np.float32)], axis=0
    ).astype(bf16)
    ident = np.eye(128, dtype=bf16)
    in_maps = []
    for c in range(N_CORES):
        er = np.roll(e, -c * R, axis=0)
        # [8192, 128] -> [128(p), 64(j)*128(d)]: row j*128+p -> [p, j, :]
        et = np.ascontiguousarray(
            er.reshape(NT, 128, D).transpose(1, 0, 2).reshape(128, NT * D)
        )
        in_maps.append({"e": et, "ident": ident})
    res = run_bass_kernel_spmd(nc, in_maps, list(range(N_CORES)), trace=_trace)
    _CACHE["last_exec_time_ns"] = res.exec_time_ns
    _CACHE["last_res"] = res

    rs = np.empty(TB, np.float64)
    z2 = np.empty(TB, np.float64)
    pos = np.empty(TB, np.float64)
    for c in range(N_CORES):
        o = res.results[c]
        rows = slice(c * R, (c + 1) * R)
        # out[p, m] is local row m*128+p -> transpose to row-major order.
        rs[rows] = o["rs"].astype(np.float64).T.reshape(-1)
        z2[rows] = o["z2"].astype(np.float64).T.reshape(-1)
        pos[rows] = o["pos"].astype(np.float64).T.reshape(-1)

    den = rs - np.exp(z2 / TAU)
    loss = np.mean(np.log(den) - pos / TAU)
    return np.float32(loss)


# revision 11
# speedup vs baseline: 1.4829x; 1.3627x over previous
"""NT-Xent loss Trainium2 kernel, symmetric/circulant variant (8-core SPMD).

sim = Z Z^T is SYMMETRIC: row sums of exp(sim/tau) only need the upper
circulant half. Each core (rotation makes local rows 0..1023 its own)
computes blocks (a, b) for its 8 row-tiles a and b = a+k, k = 0..32 --
tiles 0..39 of the rotated matrix only (2.5MiB in, 40 transposes, and
HALF the exp work of the full-matrix kernel: 264 vs 512 tile-exps).

Per block (a,b): ACT exps the PSUM sim batch into SBUF (bf16) with
accum_out giving this a-row's partial row sums. The PE then multiplies
ones^T @ E (ldweights=ones) accumulating per-COLUMN sums into a PSUM
colacc window per 8-tile octet -- those are the row-sum contributions of
the mirrored blocks (b, a), output per-core as colpart and scatter-added
on the host (np.roll). Blocks at k=32 are computed by BOTH endpoint
cores (the circulant wraps), each contributing row sums only. The k=0
diagonal block's colsums are excluded (its rows are fully in rowp).
Colaccs are DVE-zeroed and accumulated with start=False ONLY: pure
accumulates commute, so the list scheduler cannot corrupt them
(start=True resets interleaved with accumulates DO get misordered).

Host: den_r = rowp_r + colp_r - exp(z2_r/tau);
      loss = mean(log den - pos/tau).
"""

import numpy as np

B = 4096
TB = 2 * B
D = 128
TAU = 0.5
N_CORES = 8
R = TB // N_CORES   # 1024 rows per core
MT = R // 128       # 8 row-tiles owned per core
NT = TB // 128      # 64 row-tiles total
NO = 5              # octets of column tiles held per core (tiles 0..39)
CT = NO * 8         # 40 column tiles
MAGIC = 0x5F3759DF

_CACHE = {}


def _build():
    import concourse.tile as tile
    from concourse import bacc, mybir

    f32 = mybir.dt.float32
    bf16 = mybir.dt.bfloat16
    i32 = mybir.dt.int32
    Exp = mybir.ActivationFunctionType.Exp
    OpAdd = mybir.AluOpType.add
    OpMult = mybir.AluOpType.mult
    OpShr = mybir.AluOpType.arith_shift_right
    OpXor = mybir.AluOpType.bitwise_xor
    AxisX = mybir.AxisListType.X

    nc = bacc.Bacc(
        "TRN2", target_bir_lowering=False, debug=False, num_devices=N_CORES
    )
    # host pre-rotated AND pre-transposed: [128(p), 64(j)*128(d)]
    e_ap = nc.dram_tensor("e", [128, NT * D], bf16, kind="ExternalInput").ap()
    ident_ap = nc.dram_tensor("ident", [128, 128], bf16, kind="ExternalInput").ap()
    ones_ap = nc.dram_tensor("ones", [128, 128], bf16, kind="ExternalInput").ap()
    rs_ap = nc.dram_tensor("rs", [128, MT], f32, kind="ExternalOutput").ap()
    cp_ap = nc.dram_tensor("colp", [1, CT * 128], f32, kind="ExternalOutput").ap()
    pos_ap = nc.dram_tensor("pos", [128, MT], f32, kind="ExternalOutput").ap()
    z2_ap = nc.dram_tensor("z2", [128, MT], f32, kind="ExternalOutput").ap()

    def chunks512(lo, hi):
        # matmul output must not cross a PSUM bank (512 f32 cols)
        j = lo
        while j < hi:
            je = min((j // 512 + 1) * 512, hi)
            yield j, je
            j = je

    with tile.TileContext(nc) as tc:
        with (
            tc.tile_pool(name="xp", bufs=1) as xp,
            tc.tile_pool(name="ztp", bufs=1) as ztp,
            tc.tile_pool(name="small", bufs=1) as sp,
            tc.tile_pool(name="sq", bufs=2) as sqp,
            tc.tile_pool(name="ep", bufs=2) as ep,
            tc.tile_pool(name="ps", bufs=2, space="PSUM") as pp,
            tc.tile_pool(name="ca", bufs=2, space="PSUM") as cap,
        ):
            ident = sp.tile([128, 128], bf16, tag="ident")
            nc.scalar.dma_start(ident[:], ident_ap[:])
            ones = sp.tile([128, 128], bf16, tag="ones")
            nc.scalar.dma_start(ones[:], ones_ap[:])
            dummy = sp.tile([128, 1], f32, tag="dummy")
            nc.scalar.activation(dummy[:], ident[:, 0:1], Exp)

            # Input DMA on the sync queue in octet order; octet 0 split in
            # halves so its prep chain starts ~1us earlier.
            xgs = [
                xp.tile([128, 1024], bf16, tag=f"xg{o}", name=f"xg{o}")
                for o in range(NO)
            ]
            # tiles 4-7 first: o0 blocks [a..7] all need tile 7, and with
            # descending a the first block needs ONLY tile 7.
            nc.sync.dma_start(xgs[0][:, 512:1024], e_ap[:, 512:1024])
            nc.sync.dma_start(xgs[0][:, 0:512], e_ap[:, 0:512])
            for o in range(1, NO):
                nc.sync.dma_start(xgs[o][:], e_ap[:, o * 1024 : (o + 1) * 1024])

            def xtile(t):
                o, j = divmod(t, 8)
                return xgs[o][:, j * 128 : (j + 1) * 128]

            s2 = sp.tile([128, CT], f32, tag="s2")
            inv = sp.tile([128, CT], f32, tag="inv")
            nrt = sp.tile([128, CT], f32, tag="nrt")
            rsparts = sp.tile([128, MT * NO], f32, tag="rsparts")
            rs_t = sp.tile([128, MT], f32, tag="rs")
            pos_t = sp.tile([128, MT], f32, tag="pos")
            z2_t = sp.tile([128, MT], f32, tag="z2")
            inv2 = sp.tile([128, MT], f32, tag="inv2")
            colpart = sp.tile([128, CT * 128], f32, tag="colpart")

            zts = [
                ztp.tile([128, 1024], bf16, tag=f"zt{o}", name=f"zt{o}")
                for o in range(NO)
            ]

            def zttile(t):
                o, j = divmod(t, 8)
                return zts[o][:, j * 128 : (j + 1) * 128]

            def rsqrt(cols):
                s2i = s2[:, cols].bitcast(i32)
                invi = inv[:, cols].bitcast(i32)
                nc.vector.tensor_scalar(
                    out=invi, in0=s2i, scalar1=1, scalar2=-1,
                    op0=OpShr, op1=OpXor,
                )
                nc.vector.tensor_scalar(
                    out=invi, in0=invi, scalar1=MAGIC + 1, scalar2=None, op0=OpAdd
                )
                nr = nrt[:, cols]
                nc.vector.tensor_tensor(nr, inv[:, cols], inv[:, cols], OpMult)
                nc.vector.tensor_tensor(nr, nr, s2[:, cols], OpMult)
                nc.vector.tensor_scalar(
                    out=nr, in0=nr, scalar1=-0.5, scalar2=1.5,
                    op0=OpMult, op1=OpAdd,
                )
                nc.vector.tensor_tensor(inv[:, cols], inv[:, cols], nr, OpMult)

            def prep_dve(o, j0=0, nj=8):
                gcols = slice(o * 8 + j0, o * 8 + j0 + nj)
                sq = sqp.tile([128, nj * 128], f32, tag="sq", name=f"sq{o}_{j0}")
                xs = xgs[o][:, j0 * 128 : (j0 + nj) * 128]
                nc.vector.tensor_tensor(sq[:], xs, xs, OpMult)
                sq3 = sq[:].rearrange("p (j d) -> p j d", d=128)
                nc.vector.tensor_reduce(s2[:, gcols], sq3, axis=AxisX, op=OpAdd)
                rsqrt(gcols)
                for j in range(nj):
                    t = o * 8 + j0 + j
                    nc.vector.tensor_scalar_mul(xtile(t), xtile(t), inv[:, t : t + 1])

            def prep_tp(o, j0, cn, ncopy=1):
                tp = pp.tile([128, 1024], bf16, tag="ps", name=f"tp{o}_{j0}")
                for j in range(cn):
                    nc.tensor.transpose(
                        tp[:, j * 128 : (j + 1) * 128], xtile(o * 8 + j0 + j),
                        ident[:],
                    )
                cc = cn // ncopy
                for k in range(0, cn, cc):
                    nc.vector.tensor_copy(
                        zts[o][:, (j0 + k) * 128 : (j0 + k + cc) * 128],
                        tp[:, k * 128 : (k + cc) * 128],
                    )

            def block(o, a, colacc):
                """Process pair-batch (octet o, row-tile a)."""
                blo = max(a, o * 8)
                bhi = min(a + 32, o * 8 + 7)
                nb = bhi - blo + 1
                c0 = (blo - o * 8) * 128
                cw = nb * 128
                mm = pp.tile([128, 1024], f32, tag="ps", name=f"mm{o}_{a}")
                lhsT = zttile(a)
                for j, je in chunks512(c0, c0 + cw):
                    nc.tensor.matmul(mm[:, j:je], lhsT, zts[o][:, j:je])
                et = ep.tile([128, 1024], bf16, tag="e", name=f"e{o}_{a}")
                nc.scalar.activation(
                    et[:, c0 : c0 + cw], mm[:, c0 : c0 + cw], Exp,
                    scale=1.0 / TAU,
                    accum_out=rsparts[:, a * NO + o : a * NO + o + 1],
                )
                # Excluded from colsums: the k==32 tile (o==4 last tile;
                # row-accum on both endpoint cores) and the k==0 diagonal
                # (o==0 first tile; its colsums equal its rowp entry).
                clo = c0 + 128 if o == 0 else 0
                chi = c0 + cw if o < 4 else a * 128
                for j, je in chunks512(clo, chi):
                    nc.tensor.matmul(
                        colacc[:, j:je], ones[:], et[:, j:je],
                        start=False, stop=True,
                    )

            # scheduling anchors (sim ms) -- SPARSE: only to hold next-octet
            # prep out of the current octet's early window. (Dense anchoring
            # rate-limits the real schedule -- measured, do not do it.)
            OSTART = [0.0045, 0.0115, 0.0215, 0.0315, 0.0405]

            # octet-0 prep in halves (ramp-critical), tiles 4-7 first
            prep_dve(0, 4, 4)
            prep_tp(0, 4, 4, ncopy=2)
            prep_dve(0, 0, 4)
            prep_tp(0, 0, 4, ncopy=2)

            for o in range(NO):
                if o == 1:
                    with tc.tile_wait_until(OSTART[1]):
                        nc.vector.tensor_tensor(
                            inv2[:], inv[:, :MT], inv[:, :MT], OpMult
                        )
                        nc.vector.tensor_tensor(
                            z2_t[:], inv2[:], s2[:, :MT], OpMult
                        )
                if o == 4:
                    with tc.tile_wait_until(OSTART[4]):
                        psq = sqp.tile([128, MT * 128], f32, tag="sq", name="psq")
                        nc.vector.tensor_tensor(
                            psq[:], xgs[0][:], xgs[4][:], OpMult
                        )
                        psq3 = psq[:].rearrange("p (m d) -> p m d", d=128)
                        nc.vector.tensor_reduce(pos_t[:], psq3, axis=AxisX, op=OpAdd)
                if o == 0:
                    colacc = cap.tile([128, 1024], f32, tag="ca", name="ca0")
                    nc.vector.memset(colacc[:], 0.0)
                else:
                    colacc = colacc_next
                aorder = range(MT - 1, -1, -1) if o == 0 else range(MT)
                for ai, a in enumerate(aorder):
                    if o < NO - 1 and ai == 0:
                        with tc.tile_wait_until(OSTART[o]):
                            prep_dve(o + 1)
                    if o < NO - 1 and ai == 5:
                        w = OSTART[o + 1] - OSTART[o]
                        with tc.tile_wait_until(OSTART[o] + 0.55 * w):
                            prep_tp(o + 1, 0, 4)
                        with tc.tile_wait_until(OSTART[o] + 0.72 * w):
                            prep_tp(o + 1, 4, 4)
                    if o < NO - 1 and ai == 6:
                        colacc_next = cap.tile(
                            [128, 1024], f32, tag="ca", name=f"ca{o + 1}"
                        )
                        with tc.tile_wait_until(
                            OSTART[o] + 0.8 * (OSTART[o + 1] - OSTART[o])
                        ):
                            nc.vector.memset(colacc_next[:], 0.0)
                    block(o, a, colacc)
                nc.vector.tensor_copy(
                    colpart[:, o * 1024 : (o + 1) * 1024], colacc[:]
                )

            rsp3 = rsparts[:].rearrange("p (a o) -> p a o", o=NO)
            nc.vector.tensor_reduce(rs_t[:], rsp3, axis=AxisX, op=OpAdd)

            nc.sync.dma_start(rs_ap[:], rs_t[:])
            nc.sync.dma_start(cp_ap[:], colpart[0:1, :])
            nc.sync.dma_start(pos_ap[:], pos_t[:])
            nc.sync.dma_start(z2_ap[:], z2_t[:])

    nc.compile()
    return nc


def _get_nc():
    if "nc" not in _CACHE:
        _CACHE["nc"] = _build()
    return _CACHE["nc"]


def kernel(e_i: np.ndarray, e_j: np.ndarray, _trace: bool = False):
    import ml_dtypes
    from concourse.bass_utils import run_bass_kernel_spmd

    bf16 = ml_dtypes.bfloat16
    nc = _get_nc()
    e = np.concatenate(
        [np.asarray(e_i, np.float32), np.asarray(e_j, np.float32)], axis=0
    ).astype(bf16)
    ident = np.eye(128, dtype=bf16)
    ones = np.ones((128, 128), dtype=bf16)
    in_maps = []
    for c in range(N_CORES):
        er = np.roll(e, -c * R, axis=0)
        et = np.ascontiguousarray(
            er.reshape(NT, 128, D).transpose(1, 0, 2).reshape(128, NT * D)
        )
        in_maps.append({"e": et, "ident": ident, "ones": ones})
    res = run_bass_kernel_spmd(nc, in_maps, list(range(N_CORES)), trace=_trace)
    _CACHE["last_exec_time_ns"] = res.exec_time_ns
    _CACHE["last_res"] = res

    rowp = np.zeros(TB, np.float64)
    colp = np.zeros(TB, np.float64)
    z2 = np.empty(TB, np.float64)
    pos = np.empty(TB, np.float64)
    for c in range(N_CORES):
        o = res.results[c]
        rows = slice(c * R, (c + 1) * R)
        rowp[rows] = o["rs"].astype(np.float64).T.reshape(-1)
        z2[rows] = o["z2"].astype(np.float64).T.reshape(-1)
        pos[rows] = o["pos"].astype(np.float64).T.reshape(-1)
        # colpart local col j -> global row (c*1024 + j) mod 8192.
        # Tile 39 is never a colsum target (k=32 is row-accum only).
        buf = np.zeros(TB, np.float64)
        buf[: 39 * 128] = o["colp"].astype(np.float64).reshape(-1)[: 39 * 128]
        colp += np.roll(buf, c * R)

    den = rowp + colp - np.exp(z2 / TAU)
    loss = np.mean(np.log(den) - pos / TAU)
    return np.float32(loss)
